# revision 32
# baseline (speedup 1.0000x reference)
"""Trainium2 Bass kernel for nn_ConvShare: multi-width causal conv + shared projection.

Reference computation (per batch element b):
    xpad = pad(x[b], L -> L+W-1)                       # [L+11, D]
    taps[k]  = xpad[k:k+L, :] @ conv_w[:, :, k].T      # [L, D], k = 0..W-1
    spans[k] = cumsum_k taps                           # [L, D]
    h[k]     = relu(spans[k])
    out[:, k, :] = h[k] @ proj_w.T + proj_b            # [L, W, D]

Sharding: data-parallel over batch B=8 across the 8 NeuronCores (no
communication; conv_w/proj_w replicated per core).

On-chip layout is feature-major ([D, L], contraction dim on SBUF
partitions) for the conv stage; the proj stage uses h as the stationary
matmul operand so its output lands row-major [L, D] and DMAs straight
into the final [L, W, D] layout with 3KB contiguous bursts.

MODE selects the matmul input dtype (PSUM accumulation is fp32 in all
modes; the conv cumsum is carried in fp32):
  - "f16" (default): fp16 inputs. Full PE rate (1 cycle/row) with fast
    weight load; ~209us/core, rel err ~4e-4. Value ranges here (|x|<~6,
    |w|<0.04, |h|<~8) are far inside fp16 range.
  - "f32r": full fp32 data in the fast fp32 PE mode. Most accurate
    (~2e-4) but each matmul pays a ~227ns 4-byte LDWEIGHTS -> ~282us.
  - "bf16": same speed as f16 but ~8x worse rounding (~3.5e-3).
"""

import os
import sys

import numpy as np

if True:  # make concourse importable regardless of harness cwd
    for _p in ("/opt/trn_rl_repo", "/opt/pypackages"):
        if _p not in sys.path and os.path.isdir(_p):
            sys.path.append(_p)

from contextlib import ExitStack  # noqa: E402

import ml_dtypes  # noqa: E402

import concourse.bacc as bacc  # noqa: E402
import concourse.bass as bass  # noqa: E402
import concourse.mybir as mybir  # noqa: E402
import concourse.tile as tile  # noqa: E402
from concourse import bass_utils  # noqa: E402

B, L, D, W = 8, 512, 768, 12
P = 128          # SBUF partitions
C = D // P       # 6 contraction chunks of 128
LP = L + W - 1   # 523: right-padded sequence length
NB = L // P      # 4 output row blocks for proj

F32 = mybir.dt.float32
RELU = mybir.ActivationFunctionType.Relu

MODE = "f16"     # "f32r" | "bf16" | "f16"
CUMSUM = "sbuf"  # "sbuf" | "psum"
STRUCT = "lmajor"  # proj output layout: "lmajor" ([l,o2], direct DMA) | "fmajor" ([o2,l], host transpose)
WARMUP = 28      # fp16 HAM warm-up matmuls (N=128, ~107ns each cold): fill the
                 # ~4us PE-idle window between the runtime prologue and the
                 # first input chunks landing, so the HAM clock gate (1.2 ->
                 # 2.4 GHz after ~3.4us of sustained PE activity) is already
                 # open when the real matmul stream starts.
OUT16 = True     # store out as fp16 (host upcasts); halves output DMA bytes/descriptors

# Knobs the test harness may flip before calling kernel():
TRACE = False
LAST_RESULTS = None


def _build_program(mode: str, cumsum: str = "sbuf", struct: str = "fmajor") -> bass.Bass:
    mdt = {
        "f32r": mybir.dt.float32r,
        "bf16": mybir.dt.bfloat16,
        "f16": mybir.dt.float16,
    }[mode]

    nc = bacc.Bacc(
        "TRN2",
        target_bir_lowering=False,
        debug=False,
        num_devices=B,
    )

    odt = mybir.dt.float16 if OUT16 else F32

    # DRAM I/O. Matmul inputs are pre-chunked host-side to [C, P, n] so each
    # chunk DMA is a clean 2D copy and compute can start on chunk 0 early.
    xT = nc.dram_tensor("xT", [C, P, LP], mdt, kind="ExternalInput").ap()
    cw = nc.dram_tensor("cw", [W, C, P, D], mdt, kind="ExternalInput").ap()
    pw = nc.dram_tensor("pw", [C, P, D], mdt, kind="ExternalInput").ap()
    if struct == "fmajor":
        pb = nc.dram_tensor("pb", [D, 1], F32, kind="ExternalInput").ap()
        out = nc.dram_tensor("out", [W, D, L], odt, kind="ExternalOutput").ap()
    else:
        pb = nc.dram_tensor("pb", [P, D], F32, kind="ExternalInput").ap()
        out = nc.dram_tensor("out", [L, W, D], odt, kind="ExternalOutput").ap()

    with tile.TileContext(nc) as tc, ExitStack() as ctx:
        const_pool = ctx.enter_context(tc.tile_pool(name="const", bufs=1))
        cw_pool = ctx.enter_context(tc.tile_pool(name="cw", bufs=3))
        h_pool = ctx.enter_context(tc.tile_pool(name="h", bufs=2))
        out_pool = ctx.enter_context(tc.tile_pool(name="out", bufs=4))
        if cumsum == "psum":
            psc_pool = ctx.enter_context(tc.tile_pool(name="psc", bufs=1, space="PSUM"))
            psp_pool = ctx.enter_context(tc.tile_pool(name="psp", bufs=2, space="PSUM"))
        else:
            psc_pool = ctx.enter_context(tc.tile_pool(name="psc", bufs=4, space="PSUM"))
            psp_pool = ctx.enter_context(tc.tile_pool(name="psp", bufs=4, space="PSUM"))

        if WARMUP:
            # Dummy matmuls with no data dependencies: they run during the
            # initial DMA wait and hold the PE busy >3.4us so the HAM clock
            # gate opens (1.2 -> 2.4 GHz) before the first real matmul. Same
            # dtype class as the real matmuls (fp32 dummies hang the HW).
            # N=128 keeps the granularity fine so the last dummy ends close
            # to when the first input chunk lands.
            wa = const_pool.tile([P, P], mdt, name="warm_a")
            nc.gpsimd.memset(wa[:], 0.0)
            for wi in range(WARMUP):
                wp = psc_pool.tile([P, P], F32, tag="psc", name=f"warm_ps{wi}")
                nc.tensor.matmul(
                    wp[:], lhsT=wa[:], rhs=wa[:], start=True, stop=True
                )

        def load_cw(k, split=False):
            # cw[1] is on the startup critical path while the sync queue is
            # still busy with cw[0]; split it across both HWDGE queues.
            ts = []
            for c in range(C):
                t = cw_pool.tile([P, D], mdt, tag=f"cw{c}", name=f"cw{c}_{k}")
                eng = nc.scalar if (split and c % 2 == 1) else nc.sync
                eng.dma_start(t[:], cw[k, c, :, :])
                ts.append(t)
            return ts

        # Startup loads on two HWDGE queues so descriptor generation runs in
        # parallel: cw[0] chunks on the Sync queue, xT chunks on the Scalar
        # queue. Pair c of (cw0[c], xT[c]) lands ~0.6us apart, so the first
        # conv matmuls start as soon as pair 0 arrives (~1.3us after the
        # runtime prologue) instead of waiting for the whole startup set.
        cw_cur = []
        xT_t = []
        for c in range(C):
            t = cw_pool.tile([P, D], mdt, tag=f"cw{c}", name=f"cw{c}_0")
            nc.sync.dma_start(t[:], cw[0, c, :, :])
            cw_cur.append(t)
            xt = const_pool.tile([P, LP], mdt, tag=f"xt{c}", name=f"xt{c}")
            nc.scalar.dma_start(xt[:], xT[c, :, :])
            xT_t.append(xt)

        # cw[1] next (split across both queues), THEN proj weights: the two
        # HWDGE queues deliver strictly in need-order, and aggregate HBM
        # bandwidth (~330 GB/s) is the startup constraint.
        cw_nx1 = load_cw(1, split=True)

        pw_t = []
        for c in range(C):
            t = const_pool.tile([P, D], mdt, tag=f"pw{c}", name=f"pw{c}")
            eng = nc.sync if c % 2 == 0 else nc.scalar
            eng.dma_start(t[:], pw[c, :, :])
            pw_t.append(t)
        if struct == "fmajor":
            pb_t = []
            for c in range(C):
                t = const_pool.tile([P, 1], F32, tag=f"pb{c}", name=f"pb{c}")
                nc.scalar.dma_start(t[:], pb[c * P : (c + 1) * P, :])
                pb_t.append(t)
        else:
            pb_t = const_pool.tile([P, D], F32)
            nc.scalar.dma_start(pb_t[:], pb[:])

        if cumsum == "psum":
            # 6 persistent PSUM banks accumulate the conv cumsum across taps.
            sp_acc = [
                psc_pool.tile([P, L], F32, tag=f"sp{ob}", name=f"sp{ob}")
                for ob in range(C)
            ]
            spans = None
        else:
            # running conv cumsum; tap 0 writes it with a copy, so no memset
            spans = const_pool.tile([P, C * L], F32)

        def conv_tap_couter(k, cw_k):
            # Startup taps with the c-loop OUTER: each (cw[k][c], xT[c]) chunk
            # pair feeds 6 matmuls (one per output block) the moment it lands,
            # so the startup runs at the DMA arrival cadence (~1.2us/pair)
            # with no PE idle. Needs 6 concurrent accumulators; proj hasn't
            # started yet, so borrow 2 banks from the psp pool.
            h_t = [h_pool.tile([P, L], mdt, tag=f"h{c}", name=f"h{c}_{k}") for c in range(C)]
            accs = [
                psc_pool.tile([P, L], F32, tag="psc", name=f"acc{k}_{i}")
                for i in range(4)
            ] + [
                psp_pool.tile([P, 512], F32, tag="psp", name=f"acc{k}_{i + 4}")
                for i in range(2)
            ]
            nv = L - k  # tap k's last k columns are right-pad zeros: skip them
            for c in range(C):
                for ob in range(C):
                    nc.tensor.matmul(
                        accs[ob][:, 0:nv],
                        lhsT=cw_k[c][:, ob * P : (ob + 1) * P],
                        rhs=xT_t[c][:, k : k + nv],
                        start=(c == 0),
                        stop=(c == C - 1),
                    )
            for ob in range(C):
                sp = spans[:, ob * L : (ob + 1) * L]
                spv = spans[:, ob * L : ob * L + nv]
                if k == 0:
                    nc.vector.tensor_copy(sp, accs[ob][:])
                else:
                    nc.vector.tensor_add(spv, spv, accs[ob][:, 0:nv])
                nc.scalar.activation(h_t[ob][:], sp, RELU)
            return h_t

        def conv_tap(k, cw_k):
            # --- conv tap k: psum[o_blk, l] = sum_d cw^T[d, o] * x^T[d, l+k]
            if k <= 1 and cumsum == "sbuf":
                return conv_tap_couter(k, cw_k)
            h_t = [h_pool.tile([P, L], mdt, tag=f"h{c}", name=f"h{c}_{k}") for c in range(C)]
            for ob in range(C):
                if cumsum == "psum":
                    ps = sp_acc[ob]
                    for c in range(C):
                        nc.tensor.matmul(
                            ps[:],
                            lhsT=cw_k[c][:, ob * P : (ob + 1) * P],
                            rhs=xT_t[c][:, k : k + L],
                            start=(k == 0 and c == 0),
                            stop=(k == W - 1 and c == C - 1),
                            skip_group_check=True,
                        )
                    nc.scalar.activation(h_t[ob][:], ps[:], RELU)
                else:
                    ps = psc_pool.tile([P, L], F32, tag="psc")
                    nv = L - k  # tap k's last k cols are right-pad zeros
                    for c in range(C):
                        nc.tensor.matmul(
                            ps[:, 0:nv],
                            lhsT=cw_k[c][:, ob * P : (ob + 1) * P],
                            rhs=xT_t[c][:, k : k + nv],
                            start=(c == 0),
                            stop=(c == C - 1),
                        )
                    sp = spans[:, ob * L : (ob + 1) * L]
                    spv = spans[:, ob * L : ob * L + nv]
                    if k == 0:
                        nc.vector.tensor_copy(sp, ps[:])                   # init
                    else:
                        nc.vector.tensor_add(spv, spv, ps[:, 0:nv])        # cumsum
                    nc.scalar.activation(h_t[ob][:], sp, RELU)
            return h_t

        def proj_tap(k, h_t):
            if struct == "fmajor":
                # --- proj tap k (feature-major): out^T[o2_blk, l] =
                #     sum_d pw^T[d, o2] * h^T[d, l]; 36 N=512 matmuls.
                for o2b in range(C):
                    o_t = out_pool.tile([P, L], odt, tag="out", name=f"o_{k}_{o2b}")
                    pp = psp_pool.tile([P, 512], F32, tag="psp", name=f"pp_{k}_{o2b}")
                    for c in range(C):
                        nc.tensor.matmul(
                            pp[:],
                            lhsT=pw_t[c][:, o2b * P : (o2b + 1) * P],
                            rhs=h_t[c][:],
                            start=(c == 0),
                            stop=(c == C - 1),
                        )
                    nc.vector.tensor_scalar_add(o_t[:], pp[:], pb_t[o2b][:])
                    nc.scalar.dma_start(out[k, o2b * P : (o2b + 1) * P, :], o_t[:])
            else:
                # --- proj tap k: out[l_blk, o2] = sum_d h^T[d, l]*pw^T[d, o2]+b
                # On the final tap, DMA each n-chunk as soon as its bias add
                # is done (instead of one [P, D] DMA per l-block) so the tail
                # after the last matmul is one small 256-col transfer.
                split_dma = k == W - 1
                for lb in range(NB):
                    o_t = out_pool.tile([P, D], odt, tag="out")
                    for n0, nn in ((0, 512), (512, 256)):
                        pp = psp_pool.tile([P, 512], F32, tag="psp")
                        for c in range(C):
                            nc.tensor.matmul(
                                pp[:, 0:nn],
                                lhsT=h_t[c][:, lb * P : (lb + 1) * P],
                                rhs=pw_t[c][:, n0 : n0 + nn],
                                start=(c == 0),
                                stop=(c == C - 1),
                            )
                        nc.vector.tensor_add(
                            o_t[:, n0 : n0 + nn], pp[:, 0:nn], pb_t[:, n0 : n0 + nn]
                        )
                        if split_dma:
                            # alternate queues so the final descriptor-gens
                            # don't serialize on one engine
                            eng = nc.scalar if n0 == 0 else nc.sync
                            eng.dma_start(
                                out[lb * P : (lb + 1) * P, k, n0 : n0 + nn],
                                o_t[:, n0 : n0 + nn],
                            )
                    if not split_dma:
                        nc.scalar.dma_start(out[lb * P : (lb + 1) * P, k, :], o_t[:])

        # Software pipeline: issue conv(k+1) before proj(k), so proj(k)'s
        # conv->cumsum->relu dependency resolves behind conv(k+1)'s ~8us of
        # PE streaming instead of stalling the PE once per tap. cw[0]/cw[1]
        # are preloaded above; prefetch runs two taps ahead (cw bufs=3).
        h_prev = None
        for k in range(W):
            cw_nx2 = load_cw(k + 2) if k + 2 < W else None
            h_cur = conv_tap(k, cw_cur)
            if h_prev is not None:
                proj_tap(k - 1, h_prev)
            h_prev = h_cur
            cw_cur, cw_nx1 = cw_nx1, cw_nx2
        proj_tap(W - 1, h_prev)

    nc.compile()
    return nc


_program_cache: dict = {}


def _get_program(mode: str, cumsum: str = None, struct: str = None) -> bass.Bass:
    if cumsum is None:
        cumsum = CUMSUM
    if struct is None:
        struct = STRUCT
    key = (mode, cumsum, struct, WARMUP, OUT16)
    if key not in _program_cache:
        _program_cache[key] = _build_program(mode, cumsum, struct)
    return _program_cache[key]


def _np_dt(mode: str):
    return {"f32r": np.float32, "bf16": ml_dtypes.bfloat16, "f16": np.float16}[mode]


def _prep_inputs(x, conv_w, proj_w, proj_b, mode: str):
    x = np.asarray(x, dtype=np.float32)
    conv_w = np.asarray(conv_w, dtype=np.float32)
    proj_w = np.asarray(proj_w, dtype=np.float32)
    proj_b = np.asarray(proj_b, dtype=np.float32)
    ndt = _np_dt(mode)

    xT_all = np.zeros((B, D, LP), dtype=np.float32)              # [B, D, L+W-1]
    xT_all[:, :, :L] = x.transpose(0, 2, 1)
    xT_all = np.ascontiguousarray(xT_all.reshape(B, C, P, LP).astype(ndt))
    cwT = np.ascontiguousarray(
        conv_w.transpose(2, 1, 0).reshape(W, C, P, D).astype(ndt)
    )                                                            # [W, C, P, o]
    pwT = np.ascontiguousarray(proj_w.T.reshape(C, P, D).astype(ndt))
    if STRUCT == "fmajor":
        pbb = np.ascontiguousarray(proj_b.reshape(D, 1))
    else:
        pbb = np.ascontiguousarray(np.broadcast_to(proj_b[None, :], (P, D)))
    return xT_all, cwT, pwT, pbb


def kernel(x, conv_w, proj_w, proj_b):
    global LAST_RESULTS
    nc = _get_program(MODE, CUMSUM, STRUCT)
    xT_all, cwT, pwT, pbb = _prep_inputs(x, conv_w, proj_w, proj_b, MODE)
    in_maps = [
        {"xT": xT_all[b], "cw": cwT, "pw": pwT, "pb": pbb} for b in range(B)
    ]
    res = bass_utils.run_bass_kernel_spmd(
        nc, in_maps, core_ids=list(range(B)), trace=TRACE
    )
    LAST_RESULTS = res
    if STRUCT == "fmajor":
        # per-core out is [W, D, L]; final layout is [L, W, D]
        return np.stack(
            [
                np.ascontiguousarray(r["out"].transpose(2, 0, 1)).astype(np.float32)
                for r in res.results
            ],
            axis=0,
        )
    return np.stack(
        [r["out"].astype(np.float32) for r in res.results], axis=0
    )

